# revision 16
# baseline (speedup 1.0000x reference)
"""Trainium2 Bass kernel for nn_Node_GCN: out[n] = f(x[n]) + edge[n]^T @ g(cat(x,x)[n]).

Sharding: data-parallel over the batch dim N=8, one batch per NeuronCore.
Per core the dominant cost is streaming edge[n] from HBM once. edge is
carried in fp8e4 (4 MB/core, half the fp16 bytes); every 256-sender
double-tile is split as two 128-row planes, one per HWDGE ring (sync gets
even planes, scalar odd), so an unpredictably slow ring degrades all tiles
gracefully instead of starving the in-order PE queue (a starved PE also
lets the HAM clock gate re-throttle to 1.2 GHz, which is catastrophic). gx is quantized to fp8 on-chip so the edge matmuls run
in DoubleRow perf mode (2 fp8 senders per PE cell -> 256-row contraction per
pass, ~2x matmul throughput). The MLP (f and g) is interleaved into the
front of the edge matmul stream; a burst of dummy matmuls at kernel start
warms the HAM clock gate during the first DMA wait so real work runs at
2.4 GHz. Elementwise work is split between DVE (g path, output chunks 0/2)
and ACT (f path, chunks 1/3); the lazy ACT-table load attaches to the first
relu_f, which sits behind the scalar ring's DMA triggers in queue order. The four
output chunks are separate SBUF tiles (a shared tile would serialize the
two writer engines) and their stores ride the sync ring, idle by then.

The device computes outT[n] = [h, j] in fp16; the host transposes/upcasts
while unsharding.
"""

import numpy as np

D_IN = 64
D_HID = 128
M = 2048          # nodes per batch
N_BATCH = 8
NCORES = 8
NDT = 8           # edge double-tiles (256 senders each)

# fp16 blob [128, BLOB_W]: weights (rows 64:128 duplicate 0:64 for the K=64
# matmuls whose rhs lives on partitions 64:128), xT, biases.
_W_WG1 = 0          # wg1  [64, 128] (= g_w1[:64] + g_w1[64:]) dup'd
_W_FW1 = 128        # f_w1 [64, 64] dup'd
_W_FW2 = 192        # f_w2 [64, 128] dup'd
_W_GW2 = 320        # g_w2 [128, 128]
_W_XT = 448         # xT packed [128, 1024]: blob[64a + k, 448 + t] = x[1024a + t, k]
_W_GB2 = 1472       # g_b2 broadcast rows, tiled 4x along free dim [128, 512]
_W_BG1 = 1984       # g_b1 [128, 1]
_W_BF1 = 1985       # f_b1 [64, 1]
_W_BF2 = 1986       # f_b2 [128, 1]
BLOB_W = 1987

USE_DR = True       # DoubleRow fp8 edge matmuls (gx also fp8)
# rings: scalar t0-t3 (fast, early tiles), gpsimd t4 (SWDGE), sync blob+t5-t7

_NC_CACHE = {}


def _build():
    import concourse.bacc as bacc
    import concourse.mybir as mybir
    from concourse.tile import TileContext
    from concourse.bass import ts

    f32 = mybir.dt.float32
    f16 = mybir.dt.float16
    f8 = mybir.dt.float8e4
    ALU = mybir.AluOpType
    AF = mybir.ActivationFunctionType
    DR = mybir.MatmulPerfMode.DoubleRow

    nc = bacc.Bacc()
    blob_d = nc.declare_dram_parameter("blob", [128, BLOB_W], f16, isOutput=False)
    edge_d = nc.declare_dram_parameter("edge", [M, M], f8, isOutput=False)
    outT_d = nc.declare_dram_parameter("outT", [D_HID, M], f16, isOutput=True)

    NCH = M // 512   # 4 chunks of 512 for wide ops

    with TileContext(nc) as tc:
        with (
            tc.tile_pool(name="const", bufs=1) as cpool,
            tc.tile_pool(name="acts", bufs=1) as apool,
            tc.tile_pool(name="edgep", bufs=NDT) as epool,
            tc.tile_pool(name="pout", bufs=1, space="PSUM") as pout_pool,
            tc.tile_pool(name="pg", bufs=1, space="PSUM") as pg_pool,
            tc.tile_pool(name="pwork", bufs=2, space="PSUM") as pwork_pool,
        ):
            blob = cpool.tile([128, BLOB_W], f16, name="blob")
            scratch = cpool.tile([128, 512], f16, name="scratch")

            h1g = apool.tile([D_HID, M], f16, name="h1g")
            h1f = apool.tile([D_IN, M], f16, name="h1f")
            gx = apool.tile([128, 16, 128], f8 if USE_DR else f16, name="gx")
            outc = [apool.tile([128, 512], f16, name=f"outc{c}") for c in range(NCH)]
            poutA = pout_pool.tile([128, M // 2], f32, name="poutA")
            poutB = pout_pool.tile([128, M // 2], f32, name="poutB")

            def pout(c):
                t = poutA if c < 2 else poutB
                return t[:, ts(c % 2, 512)]

            ed = [epool.tile([128, 2, M], f8, tag="e", name=f"ed{i}") for i in range(NDT)]

            # --- DMA triggers -------------------------------------------------
            # ed[i][p, s, j] = edge[256i + 128s + p, j] (two 128-row planes).
            nc.sync.dma_start(out=blob, in_=blob_d[:])
            for i in range(NDT):
                nc.scalar.dma_start(out=ed[i][:, 1, :], in_=edge_d[ts(2 * i + 1, 128), :])
            for i in range(NDT):
                nc.sync.dma_start(out=ed[i][:, 0, :], in_=edge_d[ts(2 * i, 128), :])

            w_g2 = blob[0:128, _W_GW2:_W_GW2 + 128]
            gb2b4 = blob[0:128, _W_GB2:_W_GB2 + 512]

            # tensor_scalar needs fp32 scalar APs: upconvert the fp16 biases
            bias32 = cpool.tile([128, 3], f32, name="bias32")
            b_g1 = bias32[0:128, 0:1]
            b_f1 = bias32[0:64, 1:2]
            b_f2 = bias32[0:128, 2:3]

            # --- HAM warm-up: ~4.2us of dummy matmuls during the blob DMA ----
            nc.vector.memset(scratch, 0.0)
            for w in range(10):
                pw = pwork_pool.tile([128, 512], f32, tag="w", name="pwarm")
                nc.tensor.matmul(pw, scratch[:, 0:128], scratch, start=True, stop=True)

            # --- MLP pieces ---------------------------------------------------
            def mm_h1g(a):
                # tokens 1024a .. 1024a+1024 (two matmuls, one per PSUM bank)
                w = blob[64 * a:64 * a + 64, _W_WG1:_W_WG1 + 128]
                psg = pg_pool.tile([128, 1024], f32, tag="g", name="psg")
                for c2 in range(2):
                    rhs = blob[64 * a:64 * a + 64,
                               _W_XT + 512 * c2:_W_XT + 512 * c2 + 512]
                    nc.tensor.matmul(psg[:, ts(c2, 512)], w, rhs, start=True, stop=True)
                return psg, 1024 * a

            def relu_h1g(psg_tok, lo, hi, on_act=False):
                psg, tok = psg_tok
                if on_act:
                    # ACT is idle mid-stream; DVE carries the E0 chain + gx
                    nc.scalar.activation(
                        h1g[:, tok + lo:tok + hi], psg[:, lo:hi], AF.Relu, bias=b_g1)
                else:
                    nc.vector.tensor_scalar(
                        out=h1g[:, tok + lo:tok + hi], in0=psg[:, lo:hi],
                        scalar1=b_g1, scalar2=0.0, op0=ALU.add, op1=ALU.max,
                    )

            def mm_h1f(k):
                a, c2 = divmod(k, 2)
                tok = 1024 * a + 512 * c2
                w = blob[64 * a:64 * a + 64, _W_FW1:_W_FW1 + 64]
                rhs = blob[64 * a:64 * a + 64, _W_XT + 512 * c2:_W_XT + 512 * c2 + 512]
                psf = pwork_pool.tile([64, 512], f32, tag="w", name="psf")
                nc.tensor.matmul(psf, w, rhs, start=True, stop=True)
                return psf, tok

            def relu_h1f(psf_tok):
                psf, tok = psf_tok
                nc.scalar.activation(h1f[:, tok:tok + 512], psf, AF.Relu, bias=b_f1)

            def mm_gx(c, jj=range(4)):
                # gx tile j holds tokens 128j..128j+128 (matches the
                # plane-split edge DMA pairing 256i + 128s + p)
                psx = pwork_pool.tile([128, 512], f32, tag="w", name="psx")
                for kk in jj:
                    j = 4 * c + kk
                    nc.tensor.matmul(
                        psx[:, ts(kk, 128)], h1g[:, ts(j, 128)], w_g2,
                        start=True, stop=True,
                    )
                return psx

            def bias_gx(c, psx, lo, hi):
                nc.vector.tensor_add(
                    gx[:, 4 * c + lo:4 * c + hi, :],
                    psx[:, 128 * lo:128 * hi], gb2b4[:, 128 * lo:128 * hi],
                )

            def mm_sd(c):
                w = blob[0:64, _W_FW2:_W_FW2 + 128]
                nc.tensor.matmul(
                    pout(c), w, h1f[:, ts(c, 512)],
                    start=False, stop=False,
                )

            def edge_tile(i, start=False, stop=False):
                if USE_DR:
                    for c in range(NCH):
                        nc.tensor.matmul(
                            pout(c),
                            gx[:, 2 * i:2 * i + 2, :],
                            ed[i][:, :, ts(c, 512)],
                            start=start, stop=stop, perf_mode=DR,
                        )
                else:
                    for s in range(2):
                        for c in range(NCH):
                            nc.tensor.matmul(
                                pout(c),
                                gx[:, 2 * i + s, :],
                                ed[i][:, s, ts(c, 512)],
                                start=(start and s == 0), stop=(stop and s == 1),
                            )

            # --- interleaved schedule ----------------------------------------
            # chain to first edge MM: h1g a0 -> relu[0:256] -> gx 0,1 -> fp8
            # quantize -> E0. The f path and remaining g chunks are emitted
            # early so the tail (E4..E7) is purely DMA-paced. pout chunks are
            # opened by E0 (start) and closed by E7 (stop).
            nc.vector.tensor_scalar_add(bias32, blob[:, _W_BG1:_W_BG1 + 3], 0.0)
            gA = mm_h1g(0)
            relu_h1g(gA, 0, 256)
            x0 = mm_gx(0, jj=range(2))
            bias_gx(0, x0, 0, 2)
            relu_h1g(gA, 256, 512)
            x0b = mm_gx(0, jj=range(2, 4))
            bias_gx(0, x0b, 2, 4)
            edge_tile(0, start=True)
            relu_h1g(gA, 512, 1024, on_act=True)
            gB = mm_h1g(1)
            relu_h1g(gB, 0, 512, on_act=True)
            relu_h1g(gB, 512, 1024, on_act=True)
            f0 = mm_h1f(0)
            relu_h1f(f0)
            f1 = mm_h1f(1)
            relu_h1f(f1)
            edge_tile(1)
            x1 = mm_gx(1)
            bias_gx(1, x1, 0, 4)
            edge_tile(2)
            x2 = mm_gx(2)
            bias_gx(2, x2, 0, 4)
            f2 = mm_h1f(2)
            relu_h1f(f2)
            f3 = mm_h1f(3)
            relu_h1f(f3)
            edge_tile(3)
            # late-stream edge tiles are DMA-paced: keep real PE work (and a
            # couple of dummies) between them so a slow stream never idles
            # the PE long enough for the HAM clock gate to re-throttle
            mm_sd(0)
            mm_sd(1)
            edge_tile(4)
            x3 = mm_gx(3, jj=range(2))
            bias_gx(3, x3, 0, 2)
            mm_sd(2)
            edge_tile(5)
            x3b = mm_gx(3, jj=range(2, 4))
            bias_gx(3, x3b, 2, 4)
            mm_sd(3)
            edge_tile(6)
            for w in range(2):
                pw = pwork_pool.tile([128, 512], f32, tag="w", name="pfill")
                nc.tensor.matmul(pw, scratch[:, 0:128], scratch, start=True, stop=True)
            edge_tile(7, stop=True)

            # tail: bias-add copies -- DVE drains poutA (chunks 0,1), ACT
            # drains poutB (chunks 2,3) so the two engines never touch the
            # same PSUM tile; stores ride the sync ring
            for c in (0, 2, 1, 3):
                dst = outc[c]
                if c < 2:
                    nc.vector.tensor_scalar_add(dst, pout(c), b_f2)
                else:
                    nc.scalar.activation(dst, pout(c), AF.Identity, bias=b_f2)
                nc.sync.dma_start(out=outT_d[:, ts(c, 512)], in_=dst)
    nc.compile()
    return nc


def _get_nc():
    if "nc" not in _NC_CACHE:
        _NC_CACHE["nc"] = _build()
    return _NC_CACHE["nc"]


def _prep_in_maps(inputs):
    import ml_dtypes

    f8 = ml_dtypes.float8_e4m3

    x = np.asarray(inputs["x"], dtype=np.float32)
    edge = np.asarray(inputs["edge"], dtype=np.float32)
    f_w1 = np.asarray(inputs["f_w1"], dtype=np.float32)
    f_b1 = np.asarray(inputs["f_b1"], dtype=np.float32)
    f_w2 = np.asarray(inputs["f_w2"], dtype=np.float32)
    f_b2 = np.asarray(inputs["f_b2"], dtype=np.float32)
    g_w1 = np.asarray(inputs["g_w1"], dtype=np.float32)
    g_b1 = np.asarray(inputs["g_b1"], dtype=np.float32)
    g_w2 = np.asarray(inputs["g_w2"], dtype=np.float32)
    g_b2 = np.asarray(inputs["g_b2"], dtype=np.float32)

    # cat(x, x) @ g_w1 == x @ (g_w1[:64] + g_w1[64:])
    wg1 = g_w1[:D_IN] + g_w1[D_IN:]

    blob = np.zeros((128, BLOB_W), dtype=np.float16)
    for r in (slice(0, 64), slice(64, 128)):  # duplicate for partition-64 rhs
        blob[r, _W_WG1:_W_WG1 + 128] = wg1.astype(np.float16)
        blob[r, _W_FW1:_W_FW1 + 64] = f_w1.astype(np.float16)
        blob[r, _W_FW2:_W_FW2 + 128] = f_w2.astype(np.float16)
    blob[0:128, _W_GW2:_W_GW2 + 128] = g_w2.astype(np.float16)
    blob[0:128, _W_GB2:_W_GB2 + 512] = np.tile(g_b2[None, :], (128, 4)).astype(np.float16)
    blob[0:128, _W_BG1] = g_b1.astype(np.float16)
    blob[0:64, _W_BF1] = f_b1.astype(np.float16)
    blob[0:128, _W_BF2] = f_b2.astype(np.float16)

    # x[n].T packed [128, 1024]: blob[64a + k, 448 + t] = x[n, 1024a + t, k]
    xT = np.transpose(x, (0, 2, 1)).astype(np.float16)       # [8, 64, 2048]
    xT2 = np.concatenate([xT[:, :, :1024], xT[:, :, 1024:]], axis=1)  # [8, 128, 1024]

    edge8 = edge.astype(f8)
    in_maps = []
    for n in range(N_BATCH):
        b = blob.copy()
        b[:, _W_XT:_W_XT + 1024] = xT2[n]
        in_maps.append({
            "blob": b,
            "edge": np.ascontiguousarray(edge8[n]),
        })
    return in_maps


def run(inputs, trace=False, **kw):
    """Run on 8 cores; returns (out [8, 2048, 128] fp32, BassKernelResults)."""
    from concourse.bass_utils import run_bass_kernel_spmd

    nc = _get_nc()
    in_maps = _prep_in_maps(inputs)
    res = run_bass_kernel_spmd(nc, in_maps, list(range(NCORES)), trace=trace, **kw)
    outT = np.stack([np.asarray(res.results[n]["outT"]) for n in range(N_BATCH)])
    out = np.ascontiguousarray(np.transpose(outT, (0, 2, 1))).astype(np.float32)
    return out, res


def kernel(**inputs):
    out, _ = run(inputs, trace=False)
    return out


# revision 17
# speedup vs baseline: 1.1609x; 1.1609x over previous
"""Trainium2 Bass kernel for nn_Node_GCN: out[n] = f(x[n]) + edge[n]^T @ g(cat(x,x)[n]).

Sharding: data-parallel over the batch dim N=8, one batch per NeuronCore.
Per core the dominant cost is streaming edge[n] from HBM once. edge is
carried in fp8e4 (4 MB/core, half the fp16 bytes); every 256-sender
double-tile is split as two 128-row planes, one per HWDGE ring (sync gets
even planes, scalar odd), so an unpredictably slow ring degrades all tiles
gracefully instead of starving the in-order PE queue (a starved PE also
lets the HAM clock gate re-throttle to 1.2 GHz, which is catastrophic). gx is quantized to fp8 on-chip so the edge matmuls run
in DoubleRow perf mode (2 fp8 senders per PE cell -> 256-row contraction per
pass, ~2x matmul throughput). The MLP (f and g) is interleaved into the
front of the edge matmul stream; a burst of dummy matmuls at kernel start
warms the HAM clock gate during the first DMA wait so real work runs at
2.4 GHz. Elementwise work is split between DVE (g path, output chunks 0/2)
and ACT (f path, chunks 1/3); the lazy ACT-table load attaches to the first
relu_f, which sits behind the scalar ring's DMA triggers in queue order. The four
output chunks are separate SBUF tiles (a shared tile would serialize the
two writer engines) and their stores ride the sync ring, idle by then.

The device computes outT[n] = [h, j] in fp16; the host transposes/upcasts
while unsharding.
"""

import numpy as np

D_IN = 64
D_HID = 128
M = 2048          # nodes per batch
N_BATCH = 8
NCORES = 8
NDT = 8           # edge double-tiles (256 senders each)

# fp16 blob [128, BLOB_W]: weights (rows 64:128 duplicate 0:64 for the K=64
# matmuls whose rhs lives on partitions 64:128), xT, biases.
_W_WG1 = 0          # wg1  [64, 128] (= g_w1[:64] + g_w1[64:]) dup'd
_W_FW1 = 128        # f_w1 [64, 64] dup'd
_W_FW2 = 192        # f_w2 [64, 128] dup'd
_W_GW2 = 320        # g_w2 [128, 128]
_W_XT = 448         # xT packed [128, 1024]: blob[64a + k, 448 + t] = x[1024a + t, k]
_W_GB2 = 1472       # g_b2 broadcast rows, tiled 4x along free dim [128, 512]
_W_BG1 = 1984       # g_b1 [128, 1]
_W_BF1 = 1985       # f_b1 [64, 1]
_W_BF2 = 1986       # f_b2 [128, 1]
BLOB_W = 1987

USE_DR = True       # DoubleRow fp8 edge matmuls (gx also fp8)
# rings: scalar t0-t3 (fast, early tiles), gpsimd t4 (SWDGE), sync blob+t5-t7

_NC_CACHE = {}


def _build():
    import concourse.bacc as bacc
    import concourse.mybir as mybir
    from concourse.tile import TileContext
    from concourse.bass import ts

    f32 = mybir.dt.float32
    f16 = mybir.dt.float16
    f8 = mybir.dt.float8e4
    ALU = mybir.AluOpType
    AF = mybir.ActivationFunctionType
    DR = mybir.MatmulPerfMode.DoubleRow

    nc = bacc.Bacc()
    blob_d = nc.declare_dram_parameter("blob", [128, BLOB_W], f16, isOutput=False)
    edge_d = nc.declare_dram_parameter("edge", [M, M], f8, isOutput=False)
    outT_d = nc.declare_dram_parameter("outT", [D_HID, M], f16, isOutput=True)

    NCH = M // 512   # 4 chunks of 512 for wide ops

    with TileContext(nc) as tc:
        with (
            tc.tile_pool(name="const", bufs=1) as cpool,
            tc.tile_pool(name="acts", bufs=1) as apool,
            tc.tile_pool(name="edgep", bufs=NDT) as epool,
            tc.tile_pool(name="pout", bufs=1, space="PSUM") as pout_pool,
            tc.tile_pool(name="pg", bufs=1, space="PSUM") as pg_pool,
            tc.tile_pool(name="pwork", bufs=2, space="PSUM") as pwork_pool,
        ):
            blob = cpool.tile([128, BLOB_W], f16, name="blob")
            scratch = cpool.tile([128, 512], f16, name="scratch")

            h1g = apool.tile([D_HID, M], f16, name="h1g")
            h1f = apool.tile([D_IN, M], f16, name="h1f")
            gx = apool.tile([128, 16, 128], f8 if USE_DR else f16, name="gx")
            outc = [apool.tile([128, 512], f16, name=f"outc{c}") for c in range(NCH)]
            poutA = pout_pool.tile([128, M // 2], f32, name="poutA")
            poutB = pout_pool.tile([128, M // 2], f32, name="poutB")

            def pout(c):
                t = poutA if c < 2 else poutB
                return t[:, ts(c % 2, 512)]

            ed = [epool.tile([128, 2, M], f8, tag="e", name=f"ed{i}") for i in range(NDT)]

            # --- DMA triggers -------------------------------------------------
            # ed[i][p, s, j] = edge[256i + 128s + p, j] (two 128-row planes).
            nc.sync.dma_start(out=blob, in_=blob_d[:])
            for i in range(NDT):
                nc.scalar.dma_start(out=ed[i][:, 1, :], in_=edge_d[ts(2 * i + 1, 128), :])
            for i in range(NDT):
                nc.sync.dma_start(out=ed[i][:, 0, :], in_=edge_d[ts(2 * i, 128), :])

            w_g2 = blob[0:128, _W_GW2:_W_GW2 + 128]
            gb2b4 = blob[0:128, _W_GB2:_W_GB2 + 512]

            # tensor_scalar needs fp32 scalar APs: upconvert the fp16 biases
            bias32 = cpool.tile([128, 3], f32, name="bias32")
            b_g1 = bias32[0:128, 0:1]
            b_f1 = bias32[0:64, 1:2]
            b_f2 = bias32[0:128, 2:3]

            # --- HAM warm-up: ~4.2us of dummy matmuls during the blob DMA ----
            nc.vector.memset(scratch, 0.0)
            for w in range(10):
                pw = pwork_pool.tile([128, 512], f32, tag="w", name="pwarm")
                nc.tensor.matmul(pw, scratch[:, 0:128], scratch, start=True, stop=True)

            # --- MLP pieces ---------------------------------------------------
            def mm_h1g(a):
                # tokens 1024a .. 1024a+1024 (two matmuls, one per PSUM bank)
                w = blob[64 * a:64 * a + 64, _W_WG1:_W_WG1 + 128]
                psg = pg_pool.tile([128, 1024], f32, tag="g", name="psg")
                for c2 in range(2):
                    rhs = blob[64 * a:64 * a + 64,
                               _W_XT + 512 * c2:_W_XT + 512 * c2 + 512]
                    nc.tensor.matmul(psg[:, ts(c2, 512)], w, rhs, start=True, stop=True)
                return psg, 1024 * a

            def relu_h1g(psg_tok, lo, hi):
                psg, tok = psg_tok
                nc.vector.tensor_scalar(
                    out=h1g[:, tok + lo:tok + hi], in0=psg[:, lo:hi],
                    scalar1=b_g1, scalar2=0.0, op0=ALU.add, op1=ALU.max,
                )

            def mm_h1f(k):
                a, c2 = divmod(k, 2)
                tok = 1024 * a + 512 * c2
                w = blob[64 * a:64 * a + 64, _W_FW1:_W_FW1 + 64]
                rhs = blob[64 * a:64 * a + 64, _W_XT + 512 * c2:_W_XT + 512 * c2 + 512]
                psf = pwork_pool.tile([64, 512], f32, tag="w", name="psf")
                nc.tensor.matmul(psf, w, rhs, start=True, stop=True)
                return psf, tok

            def relu_h1f(psf_tok):
                psf, tok = psf_tok
                nc.scalar.activation(h1f[:, tok:tok + 512], psf, AF.Relu, bias=b_f1)

            def mm_gx(c, jj=range(4)):
                # gx tile j holds tokens 128j..128j+128 (matches the
                # plane-split edge DMA pairing 256i + 128s + p)
                psx = pwork_pool.tile([128, 512], f32, tag="w", name="psx")
                for kk in jj:
                    j = 4 * c + kk
                    nc.tensor.matmul(
                        psx[:, ts(kk, 128)], h1g[:, ts(j, 128)], w_g2,
                        start=True, stop=True,
                    )
                return psx

            def bias_gx(c, psx, lo, hi):
                nc.vector.tensor_add(
                    gx[:, 4 * c + lo:4 * c + hi, :],
                    psx[:, 128 * lo:128 * hi], gb2b4[:, 128 * lo:128 * hi],
                )

            def mm_sd(c):
                w = blob[0:64, _W_FW2:_W_FW2 + 128]
                nc.tensor.matmul(
                    pout(c), w, h1f[:, ts(c, 512)],
                    start=False, stop=False,
                )

            def edge_tile(i, start=False, stop=False):
                if USE_DR:
                    for c in range(NCH):
                        nc.tensor.matmul(
                            pout(c),
                            gx[:, 2 * i:2 * i + 2, :],
                            ed[i][:, :, ts(c, 512)],
                            start=start, stop=stop, perf_mode=DR,
                        )
                else:
                    for s in range(2):
                        for c in range(NCH):
                            nc.tensor.matmul(
                                pout(c),
                                gx[:, 2 * i + s, :],
                                ed[i][:, s, ts(c, 512)],
                                start=(start and s == 0), stop=(stop and s == 1),
                            )

            # --- interleaved schedule ----------------------------------------
            # chain to first edge MM: h1g a0 -> relu[0:256] -> gx 0,1 -> fp8
            # quantize -> E0. The f path and remaining g chunks are emitted
            # early so the tail (E4..E7) is purely DMA-paced. pout chunks are
            # opened by E0 (start) and closed by E7 (stop).
            nc.vector.tensor_scalar_add(bias32, blob[:, _W_BG1:_W_BG1 + 3], 0.0)
            gA = mm_h1g(0)
            relu_h1g(gA, 0, 256)
            x0 = mm_gx(0, jj=range(2))
            bias_gx(0, x0, 0, 2)
            relu_h1g(gA, 256, 512)
            x0b = mm_gx(0, jj=range(2, 4))
            bias_gx(0, x0b, 2, 4)
            edge_tile(0, start=True)
            relu_h1g(gA, 512, 1024)
            gB = mm_h1g(1)
            f0 = mm_h1f(0)
            relu_h1f(f0)
            f1 = mm_h1f(1)
            relu_h1f(f1)
            edge_tile(1)
            x1 = mm_gx(1)
            bias_gx(1, x1, 0, 4)
            relu_h1g(gB, 0, 512)
            edge_tile(2)
            x2 = mm_gx(2)
            bias_gx(2, x2, 0, 4)
            f2 = mm_h1f(2)
            relu_h1f(f2)
            f3 = mm_h1f(3)
            relu_h1f(f3)
            edge_tile(3)
            # late-stream edge tiles are DMA-paced: keep real PE work (and a
            # couple of dummies) between them so a slow stream never idles
            # the PE long enough for the HAM clock gate to re-throttle
            relu_h1g(gB, 512, 1024)
            mm_sd(0)
            mm_sd(1)
            edge_tile(4)
            x3 = mm_gx(3, jj=range(2))
            bias_gx(3, x3, 0, 2)
            mm_sd(2)
            edge_tile(5)
            x3b = mm_gx(3, jj=range(2, 4))
            bias_gx(3, x3b, 2, 4)
            mm_sd(3)
            edge_tile(6)
            for w in range(2):
                pw = pwork_pool.tile([128, 512], f32, tag="w", name="pfill")
                nc.tensor.matmul(pw, scratch[:, 0:128], scratch, start=True, stop=True)
            edge_tile(7, stop=True)

            # tail: bias-add copies -- DVE drains poutA (chunks 0,1), ACT
            # drains poutB (chunks 2,3) so the two engines never touch the
            # same PSUM tile; stores ride the sync ring
            for c in (0, 2, 1, 3):
                dst = outc[c]
                if c < 2:
                    nc.vector.tensor_scalar_add(dst, pout(c), b_f2)
                else:
                    nc.scalar.activation(dst, pout(c), AF.Identity, bias=b_f2)
                nc.sync.dma_start(out=outT_d[:, ts(c, 512)], in_=dst)
    nc.compile()
    return nc


def _get_nc():
    if "nc" not in _NC_CACHE:
        _NC_CACHE["nc"] = _build()
    return _NC_CACHE["nc"]


def _prep_in_maps(inputs):
    import ml_dtypes

    f8 = ml_dtypes.float8_e4m3

    x = np.asarray(inputs["x"], dtype=np.float32)
    edge = np.asarray(inputs["edge"], dtype=np.float32)
    f_w1 = np.asarray(inputs["f_w1"], dtype=np.float32)
    f_b1 = np.asarray(inputs["f_b1"], dtype=np.float32)
    f_w2 = np.asarray(inputs["f_w2"], dtype=np.float32)
    f_b2 = np.asarray(inputs["f_b2"], dtype=np.float32)
    g_w1 = np.asarray(inputs["g_w1"], dtype=np.float32)
    g_b1 = np.asarray(inputs["g_b1"], dtype=np.float32)
    g_w2 = np.asarray(inputs["g_w2"], dtype=np.float32)
    g_b2 = np.asarray(inputs["g_b2"], dtype=np.float32)

    # cat(x, x) @ g_w1 == x @ (g_w1[:64] + g_w1[64:])
    wg1 = g_w1[:D_IN] + g_w1[D_IN:]

    blob = np.zeros((128, BLOB_W), dtype=np.float16)
    for r in (slice(0, 64), slice(64, 128)):  # duplicate for partition-64 rhs
        blob[r, _W_WG1:_W_WG1 + 128] = wg1.astype(np.float16)
        blob[r, _W_FW1:_W_FW1 + 64] = f_w1.astype(np.float16)
        blob[r, _W_FW2:_W_FW2 + 128] = f_w2.astype(np.float16)
    blob[0:128, _W_GW2:_W_GW2 + 128] = g_w2.astype(np.float16)
    blob[0:128, _W_GB2:_W_GB2 + 512] = np.tile(g_b2[None, :], (128, 4)).astype(np.float16)
    blob[0:128, _W_BG1] = g_b1.astype(np.float16)
    blob[0:64, _W_BF1] = f_b1.astype(np.float16)
    blob[0:128, _W_BF2] = f_b2.astype(np.float16)

    # x[n].T packed [128, 1024]: blob[64a + k, 448 + t] = x[n, 1024a + t, k]
    xT = np.transpose(x, (0, 2, 1)).astype(np.float16)       # [8, 64, 2048]
    xT2 = np.concatenate([xT[:, :, :1024], xT[:, :, 1024:]], axis=1)  # [8, 128, 1024]

    edge8 = edge.astype(f8)
    in_maps = []
    for n in range(N_BATCH):
        b = blob.copy()
        b[:, _W_XT:_W_XT + 1024] = xT2[n]
        in_maps.append({
            "blob": b,
            "edge": np.ascontiguousarray(edge8[n]),
        })
    return in_maps


def run(inputs, trace=False, **kw):
    """Run on 8 cores; returns (out [8, 2048, 128] fp32, BassKernelResults)."""
    from concourse.bass_utils import run_bass_kernel_spmd

    nc = _get_nc()
    in_maps = _prep_in_maps(inputs)
    res = run_bass_kernel_spmd(nc, in_maps, list(range(NCORES)), trace=trace, **kw)
    outT = np.stack([np.asarray(res.results[n]["outT"]) for n in range(N_BATCH)])
    out = np.ascontiguousarray(np.transpose(outT, (0, 2, 1))).astype(np.float32)
    return out, res


def kernel(**inputs):
    out, _ = run(inputs, trace=False)
    return out
